# revision 26
# baseline (speedup 1.0000x reference)
"""Trainium2 Bass kernel for nn_Node_Transformation.

Reference semantics, for row n:
    out[n] = x[n] @ W.T + b            if node_type[n] == item_id
             emb_weight[node_type[n]]  otherwise

Only ~1/8 of rows take the linear path, so the kernel is split:

  Dense part (all rows): out_dense[n] = emb_weight[node_type[n]], computed as a
  one-hot matmul: outT[h, r] = sum_t table[t, h] * onehot[t, r], with the tiny
  table as the stationary operand and the host-built one-hot indicator
  streaming as rhs. Output is produced hid-major ("outT") so the per-group
  varying operand is the streaming one (no LDWEIGHTS churn).

  Sparse part (selected rows only): row indices where node_type == item_id are
  computed on host (metadata only); the kernel bulk-gathers just those x rows
  with dma_gather (1/8 of x traffic, ~1 Q7 descriptor-gen call per 1024 rows),
  transposes them on the PE, and computes lin = x_sel @ W.T + b into a compact
  second output. The host scatters those rows over the dense result while
  unsharding. dma_gather needs int16 indices, so each shard's x is staged as
  two half tensors (rows < 31250 and >= 31250) with rebased indices; slots are
  padded with index 0 so num_idxs is the same on every core (SPMD).

Everything on-device is bf16 (psum accumulation in f32); the correctness gate
is a scale-relative 2e-2 absmax, bf16 error is ~4e-3.

Sharding: data-parallel over N across 8 NeuronCores; weights/table replicated.
"""

import os
import numpy as np
import ml_dtypes

import concourse.bass as bass
import concourse.bacc as bacc
import concourse.mybir as mybir
from concourse.tile import TileContext
from concourse.bass_utils import run_bass_kernel_spmd
from bass_rust import add_dep_helper

# ---- problem constants (hardcoded per contest contract) ----
N = 500000
IN_CH = 256
HID = 128
NUM_T = 8
NCORES = 8
NSH = N // NCORES          # 62500 rows per core
HALF = NSH // 2            # 31250: x split so gather indices fit int16
DGRP = 512                 # dense rows per matmul group (one f32 PSUM bank)
NG = (NSH + DGRP - 1) // DGRP          # 123 dense groups
PADR = NG * DGRP                       # 62976 padded rows per core
OHC = 8192                 # one-hot columns loaded per DMA (16 dense groups)
SLABG = 16                 # dense groups per output slab (8192 cols per DMA)
SGRP = 512                 # sel rows per matmul group (4 tiles, f32 psum bank)
GCHUNK = 1024              # sel rows per dma_gather call (2 matmul groups)
GROWS = 1                  # rows fetched per gather descriptor

BF16 = ml_dtypes.bfloat16

_CACHE = {}


def _ensure_axon_profile_hook():
    """bass_utils' trace path imports antenv.axon_hooks, which this image
    lacks. Register an equivalent module backed by the axon PJRT .so so
    trace=True (or BASS_TRACE=1) works instead of crashing."""
    try:
        import antenv.axon_hooks  # noqa: F401
        return
    except ImportError:
        pass
    import sys
    import types

    hook = None
    try:
        from trn_agent_boot.trn_boot import _ntff_profile_via_ctypes

        hook = _ntff_profile_via_ctypes("/opt/axon/libaxon_pjrt.so")
    except Exception:
        hook = None
    mod = types.ModuleType("antenv.axon_hooks")
    mod.get_axon_ntff_profile_hook = lambda: hook
    mod.set_axon_ntff_profile_hook = lambda h: None
    sys.modules["antenv.axon_hooks"] = mod
    try:
        import antenv

        antenv.axon_hooks = mod
    except ImportError:
        pass


def _build(tlo: int, thi: int) -> bass.Bass:
    """tlo/thi: number of 128-row sel tiles gathered from the low/high half
    of x. Both are multiples of 8 so gathers are whole GCHUNK calls."""
    # The stock cost model grossly underestimates SWDGE descriptor-gen time
    # (0.34 ns/descriptor; ~7.5 ns measured on hardware), which makes the
    # tile scheduler place gather-dependent work far too early. Correct it
    # for the duration of this build, then restore.
    from concourse import hw_specs as _hw
    _saved = _hw.TRN2Spec.SWDGE_NS_PER_DESCRIPTOR
    _hw.TRN2Spec.SWDGE_NS_PER_DESCRIPTOR = 7.5
    try:
        return _build_inner(tlo, thi)
    finally:
        _hw.TRN2Spec.SWDGE_NS_PER_DESCRIPTOR = _saved


def _build_inner(tlo: int, thi: int) -> bass.Bass:
    nc = bacc.Bacc("TRN2")
    f32 = mybir.dt.float32
    bf16 = mybir.dt.bfloat16
    i16 = mybir.dt.int16

    tsel = tlo + thi
    ncalls = tsel * 128 // GCHUNK
    calls_lo = tlo * 128 // GCHUNK
    sgroups = tsel * 128 // SGRP

    xlo_d = nc.dram_tensor("xlo", [HALF, IN_CH], bf16, kind="ExternalInput")
    xhi_d = nc.dram_tensor("xhi", [HALF, IN_CH], bf16, kind="ExternalInput")
    id_d = nc.dram_tensor("ident", [128, 128], bf16, kind="ExternalInput")
    oh_d = nc.dram_tensor("oh", [NUM_T, PADR], bf16, kind="ExternalInput")
    idx_d = nc.dram_tensor("idx", [128, tsel * 8], i16, kind="ExternalInput")
    t2_d = nc.dram_tensor("t2", [NUM_T, HID], bf16, kind="ExternalInput")
    wt_d = nc.dram_tensor("wt", [IN_CH, HID], bf16, kind="ExternalInput")
    bb_d = nc.dram_tensor("bb", [HID, 1], f32, kind="ExternalInput")
    outT_d = nc.dram_tensor("outT", [HID, PADR], bf16, kind="ExternalOutput")
    o2T_d = nc.dram_tensor("o2T", [HID, tsel * 128], bf16, kind="ExternalOutput")

    # spread the sel compute groups through the dense loop, starting late
    # enough that the first gather (Q7 descriptor-gen ~9us) has really landed
    first_t = 28
    sel_at = sorted(set(first_t + int(round(i * (NG - 2 - first_t) / max(1, sgroups - 1)))
                        for i in range(sgroups)))
    assert len(sel_at) == sgroups

    with TileContext(nc) as tc:
        with (
            tc.tile_pool(name="singles", bufs=1) as singles,
            tc.tile_pool(name="ohp", bufs=3) as ohpool,
            tc.tile_pool(name="osl", bufs=4) as opool,
            tc.tile_pool(name="xsp", bufs=4) as xpool,
            tc.tile_pool(name="xtp", bufs=3) as xtpool,
            tc.tile_pool(name="o2p", bufs=3) as o2pool,
            tc.tile_pool(name="psd", bufs=5, space="PSUM") as psd,
            tc.tile_pool(name="pst", bufs=2, space="PSUM") as pst,
            tc.tile_pool(name="psl", bufs=1, space="PSUM") as psl,
        ):
            oh_tiles = {}
            oslab = None
            slab_g0 = 0
            gathered = {}          # call index -> xg slab tile

            def emit_gather(k):
                if k >= ncalls or k in gathered:
                    return
                xg = xpool.tile([128, GCHUNK // 128, IN_CH], bf16, tag="xg")
                src_ap = (xlo_d if k < calls_lo else xhi_d)[:]
                cols = GCHUNK // 16
                nc.gpsimd.dma_gather(
                    out_ap=xg[:],
                    in_ap=src_ap,
                    idxs_ap=idx_s[:, k * cols : (k + 1) * cols],
                    num_idxs=GCHUNK,
                    num_idxs_reg=GCHUNK,
                    elem_size=IN_CH,
                )
                gathered[k] = xg

            def emit_sel_compute(sg, anchor):
                k = sg * SGRP // GCHUNK
                j0 = (sg * SGRP - k * GCHUNK) // 128
                xg = gathered[k]
                xsT = xtpool.tile([128, 2, SGRP], bf16, tag="xsT")
                for j in range(SGRP // 128):
                    pt = pst.tile([128, 2, 128], bf16, tag="pt")
                    t1 = nc.tensor.transpose(pt[:, 0, :], xg[:, j0 + j, 0:128], ident[:])
                    t2 = nc.tensor.transpose(pt[:, 1, :], xg[:, j0 + j, 128:256], ident[:])
                    if anchor is not None:
                        add_dep_helper(t1.ins, anchor.ins, sync=False,
                                       reason="defer sel transposes behind dense")
                    nc.vector.tensor_copy(xsT[:, :, j * 128 : (j + 1) * 128], pt[:])
                if j0 + SGRP // 128 >= GCHUNK // 128:
                    del gathered[k]
                lp = psl.tile([HID, SGRP], f32, tag="lp")
                nc.tensor.matmul(out=lp[:], lhsT=wt_s[:, 0, :], rhs=xsT[:, 0, :],
                                 start=True, stop=False)
                nc.tensor.matmul(out=lp[:], lhsT=wt_s[:, 1, :], rhs=xsT[:, 1, :],
                                 start=False, stop=True)
                o2 = o2pool.tile([HID, SGRP], bf16, tag="o2")
                nc.scalar.activation(out=o2[:], in_=lp[:],
                                     func=mybir.ActivationFunctionType.Identity,
                                     bias=bb_s[:, 0:1], scale=1.0)
                nc.scalar.dma_start(out=o2T_d[:, sg * SGRP : (sg + 1) * SGRP], in_=o2[:])

            def load_oh_chunk(ci):
                if ci * OHC >= PADR or ci in oh_tiles:
                    return
                tile = ohpool.tile([NUM_T, OHC], bf16, tag="oh")
                lo = ci * OHC
                hi = min(lo + OHC, PADR)
                nc.sync.dma_start(out=tile[:, 0 : hi - lo], in_=oh_d[:, lo:hi])
                oh_tiles[ci] = tile

            # prologue: oh chunks first (dense path must start immediately),
            # then consts, then the first gather calls
            load_oh_chunk(0)
            load_oh_chunk(1)
            t2_s = singles.tile([NUM_T, HID], bf16)
            nc.sync.dma_start(out=t2_s[:], in_=t2_d[:])
            ident = singles.tile([128, 128], bf16)
            nc.sync.dma_start(out=ident[:], in_=id_d[:])
            wt_s = singles.tile([128, 2, HID], bf16)
            nc.sync.dma_start(out=wt_s[:], in_=wt_d[:].rearrange("(k c) h -> c k h", k=2))
            bb_s = singles.tile([HID, 1], f32)
            nc.sync.dma_start(out=bb_s[:], in_=bb_d[:])
            idx_s = singles.tile([128, tsel * 8], i16)
            nc.sync.dma_start(out=idx_s[:], in_=idx_d[:])
            emit_gather(0)
            emit_gather(1)
            emit_gather(2)

            sel_i = 0
            for g in range(NG):
                c0 = g * DGRP
                ci = g // (OHC // DGRP)
                if g % (OHC // DGRP) == 0:
                    load_oh_chunk(ci + 2)
                if g % SLABG == 0:
                    oslab = opool.tile([HID, SLABG * DGRP], bf16, tag="oslab")
                    slab_g0 = g

                pd = psd.tile([HID, DGRP], f32, tag="pd")
                last_mm = nc.tensor.matmul(
                    out=pd[:], lhsT=t2_s[:],
                    rhs=oh_tiles[ci][:, c0 - ci * OHC : c0 - ci * OHC + DGRP],
                    start=True, stop=True,
                )
                so = (g - slab_g0) * DGRP
                if g % 5 < 3:
                    nc.vector.tensor_copy(oslab[:, so : so + DGRP], pd[:])
                else:
                    nc.scalar.activation(out=oslab[:, so : so + DGRP], in_=pd[:],
                                         func=mybir.ActivationFunctionType.Copy)

                if g == slab_g0 + SLABG - 1 or g == NG - 1:
                    lo = slab_g0 * DGRP
                    hi = (g + 1) * DGRP
                    nc.scalar.dma_start(out=outT_d[:, lo:hi], in_=oslab[:, 0 : hi - lo])

                if sel_i < sgroups and g == sel_at[sel_i]:
                    if sel_i % (GCHUNK // SGRP) == 0:
                        emit_gather(sel_i * SGRP // GCHUNK + 3)
                    emit_sel_compute(sel_i, last_mm)
                    sel_i += 1

            while sel_i < sgroups:
                if sel_i % (GCHUNK // SGRP) == 0:
                    emit_gather(sel_i * SGRP // GCHUNK + 3)
                emit_sel_compute(sel_i, None)
                sel_i += 1

    nc.compile()
    return nc


def _pack_idx16(vals: np.ndarray, ntiles: int) -> np.ndarray:
    """Pack a region's rebased indices (padded with 0) into the dma_gather
    index layout: slot i -> [i % 16, i // 16], 16-partition pattern."""
    arr = np.zeros(ntiles * 128, dtype=np.int16)
    arr[: len(vals)] = vals
    return np.ascontiguousarray(arr.reshape(-1, 16).T)     # [16, ntiles*8]


def _prepare(inputs):
    x = np.asarray(inputs["x"])
    nt = np.asarray(inputs["node_type"]).astype(np.int64)
    item = int(np.asarray(inputs["item_id"]))
    emb = np.asarray(inputs["emb_weight"], dtype=np.float32)
    W = np.asarray(inputs["W"], dtype=np.float32)
    b = np.asarray(inputs["b"], dtype=np.float32)

    t2 = emb.astype(BF16)
    wt = np.ascontiguousarray(W.T).astype(BF16)
    bb = b.astype(np.float32).reshape(HID, 1)

    sel_los, sel_his = [], []
    max_lo = max_hi = 0
    for c in range(NCORES):
        sel = np.flatnonzero(nt[c * NSH : (c + 1) * NSH] == item)
        sel_lo = sel[sel < HALF].astype(np.int32)
        sel_hi = (sel[sel >= HALF] - HALF).astype(np.int32)
        sel_los.append(sel_lo)
        sel_his.append(sel_hi)
        max_lo = max(max_lo, len(sel_lo))
        max_hi = max(max_hi, len(sel_hi))
    # tiles per region, multiples of 8 (whole GCHUNK calls), with headroom
    tlo = max(32, -(-max_lo // 128) + 7 & ~7)
    thi = max(32, -(-max_hi // 128) + 7 & ~7)

    in_maps = []
    for c in range(NCORES):
        nt_sh = nt[c * NSH : (c + 1) * NSH]
        xb = x[c * NSH : (c + 1) * NSH].astype(BF16)

        oh = np.zeros((NUM_T, PADR), dtype=BF16)
        for t in range(NUM_T):
            oh[t, :NSH] = (nt_sh == t)

        idx16 = np.concatenate(
            [_pack_idx16(sel_los[c], tlo), _pack_idx16(sel_his[c], thi)], axis=1)
        idx16 = np.ascontiguousarray(np.tile(idx16, (8, 1)))   # [128, tsel*8]

        in_maps.append({"xlo": np.ascontiguousarray(xb[:HALF]),
                        "xhi": np.ascontiguousarray(xb[HALF:]), "oh": oh, "idx": idx16,
                        "t2": t2, "wt": wt, "bb": bb,
                        "ident": np.eye(128, dtype=BF16)})
    return tlo, thi, sel_los, sel_his, in_maps


def _run(inputs, trace=False):
    _ensure_axon_profile_hook()
    tlo, thi, sel_los, sel_his, in_maps = _prepare(inputs)
    if (tlo, thi) not in _CACHE:
        _CACHE[(tlo, thi)] = _build(tlo, thi)
    nc = _CACHE[(tlo, thi)]
    res = run_bass_kernel_spmd(nc, in_maps, core_ids=list(range(NCORES)), trace=trace)
    out = np.empty((N, HID), np.float32)
    for c in range(NCORES):
        outT = res.results[c]["outT"]          # [HID, PADR] bf16
        osh = out[c * NSH : (c + 1) * NSH]
        osh[:] = outT[:, :NSH].astype(np.float32).T
        o2T = res.results[c]["o2T"]            # [HID, tsel*128] bf16
        lo, hi = sel_los[c], sel_his[c]
        if len(lo):
            osh[lo] = o2T[:, : len(lo)].astype(np.float32).T
        if len(hi):
            base = tlo * 128
            osh[hi + HALF] = o2T[:, base : base + len(hi)].astype(np.float32).T
    return out, res


def kernel(**inputs) -> np.ndarray:
    out, _ = _run(inputs, trace=bool(os.environ.get("KERNEL_TRACE")))
    return out


# revision 27
# speedup vs baseline: 1.0210x; 1.0210x over previous
"""Trainium2 Bass kernel for nn_Node_Transformation.

Reference semantics, for row n:
    out[n] = x[n] @ W.T + b            if node_type[n] == item_id
             emb_weight[node_type[n]]  otherwise

Only ~1/8 of rows take the linear path, so the kernel is split:

  Dense part (all rows): out_dense[n] = emb_weight[node_type[n]], computed as a
  one-hot matmul: outT[h, r] = sum_t table[t, h] * onehot[t, r], with the tiny
  table as the stationary operand and the host-built one-hot indicator
  streaming as rhs. Output is produced hid-major ("outT") so the per-group
  varying operand is the streaming one (no LDWEIGHTS churn).

  Sparse part (selected rows only): row indices where node_type == item_id are
  computed on host (metadata only); the kernel bulk-gathers just those x rows
  with dma_gather (1/8 of x traffic, ~1 Q7 descriptor-gen call per 1024 rows),
  transposes them on the PE, and computes lin = x_sel @ W.T + b into a compact
  second output. The host scatters those rows over the dense result while
  unsharding. dma_gather needs int16 indices, so each shard's x is staged as
  two half tensors (rows < 31250 and >= 31250) with rebased indices; slots are
  padded with index 0 so num_idxs is the same on every core (SPMD).

Everything on-device is bf16 (psum accumulation in f32); the correctness gate
is a scale-relative 2e-2 absmax, bf16 error is ~4e-3.

Sharding: data-parallel over N across 8 NeuronCores; weights/table replicated.
"""

import os
import numpy as np
import ml_dtypes

import concourse.bass as bass
import concourse.bacc as bacc
import concourse.mybir as mybir
from concourse.tile import TileContext
from concourse.bass_utils import run_bass_kernel_spmd
from bass_rust import add_dep_helper

# ---- problem constants (hardcoded per contest contract) ----
N = 500000
IN_CH = 256
HID = 128
NUM_T = 8
NCORES = 8
NSH = N // NCORES          # 62500 rows per core
HALF = NSH // 2            # 31250: x split so gather indices fit int16
DGRP = 512                 # dense rows per matmul group (one f32 PSUM bank)
NG = (NSH + DGRP - 1) // DGRP          # 123 dense groups
PADR = NG * DGRP                       # 62976 padded rows per core
OHC = 8192                 # one-hot columns loaded per DMA (16 dense groups)
SLABG = 16                 # dense groups per output slab (8192 cols per DMA)
SGRP = 512                 # sel rows per matmul group (4 tiles, f32 psum bank)
GCHUNK = 1024              # sel rows per dma_gather call (2 matmul groups)

BF16 = ml_dtypes.bfloat16

_CACHE = {}


def _ensure_axon_profile_hook():
    """bass_utils' trace path imports antenv.axon_hooks, which this image
    lacks. Register an equivalent module backed by the axon PJRT .so so
    trace=True (or BASS_TRACE=1) works instead of crashing."""
    try:
        import antenv.axon_hooks  # noqa: F401
        return
    except ImportError:
        pass
    import sys
    import types

    hook = None
    try:
        from trn_agent_boot.trn_boot import _ntff_profile_via_ctypes

        hook = _ntff_profile_via_ctypes("/opt/axon/libaxon_pjrt.so")
    except Exception:
        hook = None
    mod = types.ModuleType("antenv.axon_hooks")
    mod.get_axon_ntff_profile_hook = lambda: hook
    mod.set_axon_ntff_profile_hook = lambda h: None
    sys.modules["antenv.axon_hooks"] = mod
    try:
        import antenv

        antenv.axon_hooks = mod
    except ImportError:
        pass


def _build(tlo: int, thi: int) -> bass.Bass:
    """tlo/thi: number of 128-row sel tiles gathered from the low/high half
    of x. Both are multiples of 8 so gathers are whole GCHUNK calls."""
    # The stock cost model grossly underestimates SWDGE descriptor-gen time
    # (0.34 ns/descriptor; ~7.5 ns measured on hardware), which makes the
    # tile scheduler place gather-dependent work far too early. Correct it
    # for the duration of this build, then restore.
    from concourse import hw_specs as _hw
    _saved = _hw.TRN2Spec.SWDGE_NS_PER_DESCRIPTOR
    _hw.TRN2Spec.SWDGE_NS_PER_DESCRIPTOR = 7.5
    try:
        return _build_inner(tlo, thi)
    finally:
        _hw.TRN2Spec.SWDGE_NS_PER_DESCRIPTOR = _saved


def _build_inner(tlo: int, thi: int) -> bass.Bass:
    nc = bacc.Bacc("TRN2")
    f32 = mybir.dt.float32
    bf16 = mybir.dt.bfloat16
    i16 = mybir.dt.int16

    tsel = tlo + thi
    ncalls = tsel * 128 // GCHUNK
    calls_lo = tlo * 128 // GCHUNK
    sgroups = tsel * 128 // SGRP

    xlo_d = nc.dram_tensor("xlo", [HALF, IN_CH], bf16, kind="ExternalInput")
    xhi_d = nc.dram_tensor("xhi", [HALF, IN_CH], bf16, kind="ExternalInput")
    id_d = nc.dram_tensor("ident", [128, 128], bf16, kind="ExternalInput")
    oh_d = nc.dram_tensor("oh", [NUM_T, PADR], bf16, kind="ExternalInput")
    idx_d = nc.dram_tensor("idx", [128, tsel * 8], i16, kind="ExternalInput")
    t2_d = nc.dram_tensor("t2", [NUM_T, HID], bf16, kind="ExternalInput")
    wt_d = nc.dram_tensor("wt", [IN_CH, HID], bf16, kind="ExternalInput")
    bb_d = nc.dram_tensor("bb", [HID, 1], f32, kind="ExternalInput")
    outT_d = nc.dram_tensor("outT", [HID, PADR], bf16, kind="ExternalOutput")
    o2T_d = nc.dram_tensor("o2T", [HID, tsel * 128], bf16, kind="ExternalOutput")

    # spread the sel compute groups through the dense loop, starting late
    # enough that the first gather (Q7 descriptor-gen ~9us) has really landed
    first_t = 28
    sel_at = sorted(set(first_t + int(round(i * (NG - 2 - first_t) / max(1, sgroups - 1)))
                        for i in range(sgroups)))
    assert len(sel_at) == sgroups

    with TileContext(nc) as tc:
        with (
            tc.tile_pool(name="singles", bufs=1) as singles,
            tc.tile_pool(name="ohp", bufs=3) as ohpool,
            tc.tile_pool(name="osl", bufs=4) as opool,
            tc.tile_pool(name="xsp", bufs=4) as xpool,
            tc.tile_pool(name="xtp", bufs=3) as xtpool,
            tc.tile_pool(name="o2p", bufs=3) as o2pool,
            tc.tile_pool(name="psd", bufs=5, space="PSUM") as psd,
            tc.tile_pool(name="pst", bufs=2, space="PSUM") as pst,
            tc.tile_pool(name="psl", bufs=1, space="PSUM") as psl,
        ):
            oh_tiles = {}
            oslab = None
            slab_g0 = 0
            gathered = {}          # call index -> xg slab tile

            def emit_gather(k):
                if k >= ncalls or k in gathered:
                    return
                xg = xpool.tile([128, GCHUNK // 128, IN_CH], bf16, tag="xg")
                src_ap = (xlo_d if k < calls_lo else xhi_d)[:]
                cols = GCHUNK // 16
                nc.gpsimd.dma_gather(
                    out_ap=xg[:],
                    in_ap=src_ap,
                    idxs_ap=idx_s[:, k * cols : (k + 1) * cols],
                    num_idxs=GCHUNK,
                    num_idxs_reg=GCHUNK,
                    elem_size=IN_CH,
                )
                gathered[k] = xg

            def emit_sel_compute(sg, anchor):
                k = sg * SGRP // GCHUNK
                j0 = (sg * SGRP - k * GCHUNK) // 128
                xg = gathered[k]
                xsT = xtpool.tile([128, 2, SGRP], bf16, tag="xsT")
                for j in range(SGRP // 128):
                    pt = pst.tile([128, 2, 128], bf16, tag="pt")
                    t1 = nc.tensor.transpose(pt[:, 0, :], xg[:, j0 + j, 0:128], ident[:])
                    t2 = nc.tensor.transpose(pt[:, 1, :], xg[:, j0 + j, 128:256], ident[:])
                    if anchor is not None:
                        add_dep_helper(t1.ins, anchor.ins, sync=False,
                                       reason="defer sel transposes behind dense")
                    nc.vector.tensor_copy(xsT[:, :, j * 128 : (j + 1) * 128], pt[:])
                if j0 + SGRP // 128 >= GCHUNK // 128:
                    del gathered[k]
                lp = psl.tile([HID, SGRP], f32, tag="lp")
                nc.tensor.matmul(out=lp[:], lhsT=wt_s[:, 0, :], rhs=xsT[:, 0, :],
                                 start=True, stop=False)
                nc.tensor.matmul(out=lp[:], lhsT=wt_s[:, 1, :], rhs=xsT[:, 1, :],
                                 start=False, stop=True)
                o2 = o2pool.tile([HID, SGRP], bf16, tag="o2")
                nc.scalar.activation(out=o2[:], in_=lp[:],
                                     func=mybir.ActivationFunctionType.Identity,
                                     bias=bb_s[:, 0:1], scale=1.0)
                nc.scalar.dma_start(out=o2T_d[:, sg * SGRP : (sg + 1) * SGRP], in_=o2[:])

            def load_oh_chunk(ci):
                if ci * OHC >= PADR or ci in oh_tiles:
                    return
                tile = ohpool.tile([NUM_T, OHC], bf16, tag="oh")
                lo = ci * OHC
                hi = min(lo + OHC, PADR)
                nc.sync.dma_start(out=tile[:, 0 : hi - lo], in_=oh_d[:, lo:hi])
                oh_tiles[ci] = tile

            # prologue: oh chunks first (dense path must start immediately),
            # then consts, then the first gather calls
            load_oh_chunk(0)
            load_oh_chunk(1)
            t2_s = singles.tile([NUM_T, HID], bf16)
            nc.sync.dma_start(out=t2_s[:], in_=t2_d[:])
            ident = singles.tile([128, 128], bf16)
            nc.sync.dma_start(out=ident[:], in_=id_d[:])
            wt_s = singles.tile([128, 2, HID], bf16)
            nc.sync.dma_start(out=wt_s[:], in_=wt_d[:].rearrange("(k c) h -> c k h", k=2))
            bb_s = singles.tile([HID, 1], f32)
            nc.sync.dma_start(out=bb_s[:], in_=bb_d[:])
            idx_s = singles.tile([128, tsel * 8], i16)
            nc.sync.dma_start(out=idx_s[:], in_=idx_d[:])
            emit_gather(0)
            emit_gather(1)
            emit_gather(2)

            sel_i = 0
            for g in range(NG):
                c0 = g * DGRP
                ci = g // (OHC // DGRP)
                if g % (OHC // DGRP) == 0:
                    load_oh_chunk(ci + 2)
                if g % SLABG == 0:
                    oslab = opool.tile([HID, SLABG * DGRP], bf16, tag="oslab")
                    slab_g0 = g

                pd = psd.tile([HID, DGRP], f32, tag="pd")
                last_mm = nc.tensor.matmul(
                    out=pd[:], lhsT=t2_s[:],
                    rhs=oh_tiles[ci][:, c0 - ci * OHC : c0 - ci * OHC + DGRP],
                    start=True, stop=True,
                )
                so = (g - slab_g0) * DGRP
                if g % 5 < 3:
                    nc.vector.tensor_copy(oslab[:, so : so + DGRP], pd[:])
                else:
                    nc.scalar.activation(out=oslab[:, so : so + DGRP], in_=pd[:],
                                         func=mybir.ActivationFunctionType.Copy)

                if g == slab_g0 + SLABG - 1 or g == NG - 1:
                    lo = slab_g0 * DGRP
                    hi = (g + 1) * DGRP
                    nc.scalar.dma_start(out=outT_d[:, lo:hi], in_=oslab[:, 0 : hi - lo])

                if sel_i < sgroups and g == sel_at[sel_i]:
                    if sel_i % (GCHUNK // SGRP) == 0:
                        emit_gather(sel_i * SGRP // GCHUNK + 3)
                    emit_sel_compute(sel_i, last_mm)
                    sel_i += 1

            while sel_i < sgroups:
                if sel_i % (GCHUNK // SGRP) == 0:
                    emit_gather(sel_i * SGRP // GCHUNK + 3)
                emit_sel_compute(sel_i, None)
                sel_i += 1

    nc.compile()
    return nc


def _pack_idx16(vals: np.ndarray, ntiles: int) -> np.ndarray:
    """Pack a region's rebased indices (padded with 0) into the dma_gather
    index layout: slot i -> [i % 16, i // 16], 16-partition pattern."""
    arr = np.zeros(ntiles * 128, dtype=np.int16)
    arr[: len(vals)] = vals
    return np.ascontiguousarray(arr.reshape(-1, 16).T)     # [16, ntiles*8]


def _prepare(inputs):
    x = np.asarray(inputs["x"])
    nt = np.asarray(inputs["node_type"]).astype(np.int64)
    item = int(np.asarray(inputs["item_id"]))
    emb = np.asarray(inputs["emb_weight"], dtype=np.float32)
    W = np.asarray(inputs["W"], dtype=np.float32)
    b = np.asarray(inputs["b"], dtype=np.float32)

    t2 = emb.astype(BF16)
    wt = np.ascontiguousarray(W.T).astype(BF16)
    bb = b.astype(np.float32).reshape(HID, 1)

    sel_los, sel_his = [], []
    max_lo = max_hi = 0
    for c in range(NCORES):
        sel = np.flatnonzero(nt[c * NSH : (c + 1) * NSH] == item)
        sel_lo = sel[sel < HALF].astype(np.int32)
        sel_hi = (sel[sel >= HALF] - HALF).astype(np.int32)
        sel_los.append(sel_lo)
        sel_his.append(sel_hi)
        max_lo = max(max_lo, len(sel_lo))
        max_hi = max(max_hi, len(sel_hi))
    # tiles per region, multiples of 8 (whole GCHUNK calls), with headroom
    tlo = max(32, -(-max_lo // 128) + 7 & ~7)
    thi = max(32, -(-max_hi // 128) + 7 & ~7)

    in_maps = []
    for c in range(NCORES):
        nt_sh = nt[c * NSH : (c + 1) * NSH]
        xb = x[c * NSH : (c + 1) * NSH].astype(BF16)

        oh = np.zeros((NUM_T, PADR), dtype=BF16)
        for t in range(NUM_T):
            oh[t, :NSH] = (nt_sh == t)

        idx16 = np.concatenate(
            [_pack_idx16(sel_los[c], tlo), _pack_idx16(sel_his[c], thi)], axis=1)
        idx16 = np.ascontiguousarray(np.tile(idx16, (8, 1)))   # [128, tsel*8]

        in_maps.append({"xlo": np.ascontiguousarray(xb[:HALF]),
                        "xhi": np.ascontiguousarray(xb[HALF:]), "oh": oh, "idx": idx16,
                        "t2": t2, "wt": wt, "bb": bb,
                        "ident": np.eye(128, dtype=BF16)})
    return tlo, thi, sel_los, sel_his, in_maps


def _run(inputs, trace=False):
    _ensure_axon_profile_hook()
    tlo, thi, sel_los, sel_his, in_maps = _prepare(inputs)
    if (tlo, thi) not in _CACHE:
        _CACHE[(tlo, thi)] = _build(tlo, thi)
    nc = _CACHE[(tlo, thi)]
    res = run_bass_kernel_spmd(nc, in_maps, core_ids=list(range(NCORES)), trace=trace)
    out = np.empty((N, HID), np.float32)
    for c in range(NCORES):
        outT = res.results[c]["outT"]          # [HID, PADR] bf16
        osh = out[c * NSH : (c + 1) * NSH]
        osh[:] = outT[:, :NSH].astype(np.float32).T
        o2T = res.results[c]["o2T"]            # [HID, tsel*128] bf16
        lo, hi = sel_los[c], sel_his[c]
        if len(lo):
            osh[lo] = o2T[:, : len(lo)].astype(np.float32).T
        if len(hi):
            base = tlo * 128
            osh[hi + HALF] = o2T[:, base : base + len(hi)].astype(np.float32).T
    return out, res


def kernel(**inputs) -> np.ndarray:
    out, _ = _run(inputs, trace=bool(os.environ.get("KERNEL_TRACE")))
    return out


# revision 28
# speedup vs baseline: 1.0446x; 1.0231x over previous
"""Trainium2 Bass kernel for nn_Node_Transformation.

Reference semantics, for row n:
    out[n] = x[n] @ W.T + b            if node_type[n] == item_id
             emb_weight[node_type[n]]  otherwise

Only ~1/8 of rows take the linear path, so the kernel is split:

  Dense part (all rows): out_dense[n] = emb_weight[node_type[n]], computed as a
  one-hot matmul: outT[h, r] = sum_t table[t, h] * onehot[t, r], with the tiny
  table as the stationary operand and the host-built one-hot indicator
  streaming as rhs. Output is produced hid-major ("outT") so the per-group
  varying operand is the streaming one (no LDWEIGHTS churn).

  Sparse part (selected rows only): row indices where node_type == item_id are
  computed on host (metadata only); the kernel bulk-gathers just those x rows
  with dma_gather (1/8 of x traffic, ~1 Q7 descriptor-gen call per 1024 rows),
  transposes them on the PE, and computes lin = x_sel @ W.T + b into a compact
  second output. The host scatters those rows over the dense result while
  unsharding. dma_gather needs int16 indices, so each shard's x is staged as
  two half tensors (rows < 31250 and >= 31250) with rebased indices; slots are
  padded with index 0 so num_idxs is the same on every core (SPMD).

Everything on-device is bf16 (psum accumulation in f32); the correctness gate
is a scale-relative 2e-2 absmax, bf16 error is ~4e-3.

Sharding: data-parallel over N across 8 NeuronCores; weights/table replicated.
"""

import os
import numpy as np
import ml_dtypes

import concourse.bass as bass
import concourse.bacc as bacc
import concourse.mybir as mybir
from concourse.tile import TileContext
from concourse.bass_utils import run_bass_kernel_spmd
from bass_rust import add_dep_helper

# ---- problem constants (hardcoded per contest contract) ----
N = 500000
IN_CH = 256
HID = 128
NUM_T = 8
NCORES = 8
NSH = N // NCORES          # 62500 rows per core
HALF = NSH // 2            # 31250: x split so gather indices fit int16
DGRP = 512                 # dense rows per matmul group (one f32 PSUM bank)
NG = (NSH + DGRP - 1) // DGRP          # 123 dense groups
PADR = NG * DGRP                       # 62976 padded rows per core
OHC = 8192                 # one-hot columns loaded per DMA (16 dense groups)
SLABG = 16                 # dense groups per output slab (8192 cols per DMA)
SGRP = 512                 # sel rows per matmul group (4 tiles, f32 psum bank)
GCHUNK = 1024              # sel rows per dma_gather call (2 matmul groups)

BF16 = ml_dtypes.bfloat16

_CACHE = {}


def _ensure_axon_profile_hook():
    """bass_utils' trace path imports antenv.axon_hooks, which this image
    lacks. Register an equivalent module backed by the axon PJRT .so so
    trace=True (or BASS_TRACE=1) works instead of crashing."""
    try:
        import antenv.axon_hooks  # noqa: F401
        return
    except ImportError:
        pass
    import sys
    import types

    hook = None
    try:
        from trn_agent_boot.trn_boot import _ntff_profile_via_ctypes

        hook = _ntff_profile_via_ctypes("/opt/axon/libaxon_pjrt.so")
    except Exception:
        hook = None
    mod = types.ModuleType("antenv.axon_hooks")
    mod.get_axon_ntff_profile_hook = lambda: hook
    mod.set_axon_ntff_profile_hook = lambda h: None
    sys.modules["antenv.axon_hooks"] = mod
    try:
        import antenv

        antenv.axon_hooks = mod
    except ImportError:
        pass


def _build(tlo: int, thi: int) -> bass.Bass:
    """tlo/thi: number of 128-row sel tiles gathered from the low/high half
    of x. Both are multiples of 8 so gathers are whole GCHUNK calls."""
    # The stock cost model grossly underestimates SWDGE descriptor-gen time
    # (0.34 ns/descriptor; ~7.5 ns measured on hardware), which makes the
    # tile scheduler place gather-dependent work far too early. Correct it
    # for the duration of this build, then restore.
    from concourse import hw_specs as _hw
    _saved = _hw.TRN2Spec.SWDGE_NS_PER_DESCRIPTOR
    _hw.TRN2Spec.SWDGE_NS_PER_DESCRIPTOR = 7.5
    try:
        return _build_inner(tlo, thi)
    finally:
        _hw.TRN2Spec.SWDGE_NS_PER_DESCRIPTOR = _saved


def _build_inner(tlo: int, thi: int) -> bass.Bass:
    nc = bacc.Bacc("TRN2")
    f32 = mybir.dt.float32
    bf16 = mybir.dt.bfloat16
    i16 = mybir.dt.int16

    tsel = tlo + thi
    ncalls = tsel * 128 // GCHUNK
    calls_lo = tlo * 128 // GCHUNK
    sgroups = tsel * 128 // SGRP

    xlo_d = nc.dram_tensor("xlo", [HALF, IN_CH], bf16, kind="ExternalInput")
    xhi_d = nc.dram_tensor("xhi", [HALF, IN_CH], bf16, kind="ExternalInput")
    id_d = nc.dram_tensor("ident", [128, 128], bf16, kind="ExternalInput")
    oh_d = nc.dram_tensor("oh", [2, NUM_T, PADR], bf16, kind="ExternalInput")
    idx_d = nc.dram_tensor("idx", [128, tsel * 8], i16, kind="ExternalInput")
    t2_d = nc.dram_tensor("t2", [2, NUM_T, HID], bf16, kind="ExternalInput")
    wt_d = nc.dram_tensor("wt", [IN_CH, HID], bf16, kind="ExternalInput")
    bb_d = nc.dram_tensor("bb", [HID, 1], f32, kind="ExternalInput")
    outT_d = nc.dram_tensor("outT", [HID, PADR], bf16, kind="ExternalOutput")
    o2T_d = nc.dram_tensor("o2T", [HID, tsel * 128], bf16, kind="ExternalOutput")

    # spread the sel compute groups through the dense loop, starting late
    # enough that the first gather (Q7 descriptor-gen ~9us) has really landed
    first_t = 28
    sel_at = sorted(set(first_t + int(round(i * (NG - 2 - first_t) / max(1, sgroups - 1)))
                        for i in range(sgroups)))
    assert len(sel_at) == sgroups

    with TileContext(nc) as tc:
        with (
            tc.tile_pool(name="singles", bufs=1) as singles,
            tc.tile_pool(name="ohp", bufs=3) as ohpool,
            tc.tile_pool(name="osl", bufs=4) as opool,
            tc.tile_pool(name="xsp", bufs=4) as xpool,
            tc.tile_pool(name="xtp", bufs=3) as xtpool,
            tc.tile_pool(name="o2p", bufs=3) as o2pool,
            tc.tile_pool(name="psd", bufs=5, space="PSUM") as psd,
            tc.tile_pool(name="pst", bufs=2, space="PSUM") as pst,
            tc.tile_pool(name="psl", bufs=1, space="PSUM") as psl,
        ):
            oh_tiles = {}
            oslab = None
            slab_g0 = 0
            gathered = {}          # call index -> xg slab tile

            def emit_gather(k):
                if k >= ncalls or k in gathered:
                    return
                xg = xpool.tile([128, GCHUNK // 128, IN_CH], bf16, tag="xg")
                src_ap = (xlo_d if k < calls_lo else xhi_d)[:]
                cols = GCHUNK // 16
                nc.gpsimd.dma_gather(
                    out_ap=xg[:],
                    in_ap=src_ap,
                    idxs_ap=idx_s[:, k * cols : (k + 1) * cols],
                    num_idxs=GCHUNK,
                    num_idxs_reg=GCHUNK,
                    elem_size=IN_CH,
                )
                gathered[k] = xg

            def emit_sel_compute(sg, anchor):
                k = sg * SGRP // GCHUNK
                j0 = (sg * SGRP - k * GCHUNK) // 128
                xg = gathered[k]
                xsT = xtpool.tile([128, 2, SGRP], bf16, tag="xsT")
                for j in range(SGRP // 128):
                    pt = pst.tile([128, 2, 128], bf16, tag="pt")
                    t1 = nc.tensor.transpose(pt[:, 0, :], xg[:, j0 + j, 0:128], ident[:])
                    t2 = nc.tensor.transpose(pt[:, 1, :], xg[:, j0 + j, 128:256], ident[:])
                    if anchor is not None:
                        add_dep_helper(t1.ins, anchor.ins, sync=False,
                                       reason="defer sel transposes behind dense")
                    nc.vector.tensor_copy(xsT[:, :, j * 128 : (j + 1) * 128], pt[:])
                if j0 + SGRP // 128 >= GCHUNK // 128:
                    del gathered[k]
                lp = psl.tile([HID, SGRP], f32, tag="lp")
                nc.tensor.matmul(out=lp[:], lhsT=wt_s[:, 0, :], rhs=xsT[:, 0, :],
                                 start=True, stop=False)
                nc.tensor.matmul(out=lp[:], lhsT=wt_s[:, 1, :], rhs=xsT[:, 1, :],
                                 start=False, stop=True)
                o2 = o2pool.tile([HID, SGRP], bf16, tag="o2")
                nc.scalar.activation(out=o2[:], in_=lp[:],
                                     func=mybir.ActivationFunctionType.Identity,
                                     bias=bb_s[:, 0:1], scale=1.0)
                nc.scalar.dma_start(out=o2T_d[:, sg * SGRP : (sg + 1) * SGRP], in_=o2[:])

            def load_oh_chunk(ci):
                if ci * OHC >= PADR or ci in oh_tiles:
                    return
                tile = ohpool.tile([64, OHC], bf16, tag="oh")
                lo = ci * OHC
                hi = min(lo + OHC, PADR)
                for a in range(2):
                    nc.sync.dma_start(out=tile[32 * a : 32 * a + NUM_T, 0 : hi - lo],
                                      in_=oh_d[a, :, lo:hi])
                oh_tiles[ci] = tile

            # prologue: oh chunks first (dense path must start immediately),
            # then consts, then the first gather calls
            load_oh_chunk(0)
            load_oh_chunk(1)
            t2_s = singles.tile([64, HID], bf16)
            for a in range(2):
                nc.sync.dma_start(out=t2_s[32 * a : 32 * a + NUM_T, :], in_=t2_d[a])
            ident = singles.tile([128, 128], bf16)
            nc.sync.dma_start(out=ident[:], in_=id_d[:])
            wt_s = singles.tile([128, 2, HID], bf16)
            nc.sync.dma_start(out=wt_s[:], in_=wt_d[:].rearrange("(k c) h -> c k h", k=2))
            bb_s = singles.tile([HID, 1], f32)
            nc.sync.dma_start(out=bb_s[:], in_=bb_d[:])
            idx_s = singles.tile([128, tsel * 8], i16)
            nc.sync.dma_start(out=idx_s[:], in_=idx_d[:])
            emit_gather(0)
            emit_gather(1)
            emit_gather(2)

            sel_i = 0
            for g in range(NG):
                c0 = g * DGRP
                ci = g // (OHC // DGRP)
                if g % (OHC // DGRP) == 0:
                    load_oh_chunk(ci + 2)
                if g % SLABG == 0:
                    oslab = opool.tile([HID, SLABG * DGRP], bf16, tag="oslab")
                    slab_g0 = g

                j = g % 2
                pd = psd.tile([HID, DGRP], f32, tag="pd")
                last_mm = nc.tensor.matmul(
                    out=pd[:], lhsT=t2_s[32 * j : 32 * j + NUM_T, :],
                    rhs=oh_tiles[ci][32 * j : 32 * j + NUM_T,
                                     c0 - ci * OHC : c0 - ci * OHC + DGRP],
                    start=True, stop=True,
                    tile_position=(32 * j, 0),
                )
                so = (g - slab_g0) * DGRP
                if g % 5 < 3:
                    nc.vector.tensor_copy(oslab[:, so : so + DGRP], pd[:])
                else:
                    nc.scalar.activation(out=oslab[:, so : so + DGRP], in_=pd[:],
                                         func=mybir.ActivationFunctionType.Copy)

                if g == slab_g0 + SLABG - 1 or g == NG - 1:
                    lo = slab_g0 * DGRP
                    hi = (g + 1) * DGRP
                    nc.scalar.dma_start(out=outT_d[:, lo:hi], in_=oslab[:, 0 : hi - lo])

                if sel_i < sgroups and g == sel_at[sel_i]:
                    if sel_i % (GCHUNK // SGRP) == 0:
                        emit_gather(sel_i * SGRP // GCHUNK + 3)
                    emit_sel_compute(sel_i, last_mm)
                    sel_i += 1

            while sel_i < sgroups:
                if sel_i % (GCHUNK // SGRP) == 0:
                    emit_gather(sel_i * SGRP // GCHUNK + 3)
                emit_sel_compute(sel_i, None)
                sel_i += 1

    nc.compile()
    return nc


def _pack_idx16(vals: np.ndarray, ntiles: int) -> np.ndarray:
    """Pack a region's rebased indices (padded with 0) into the dma_gather
    index layout: slot i -> [i % 16, i // 16], 16-partition pattern."""
    arr = np.zeros(ntiles * 128, dtype=np.int16)
    arr[: len(vals)] = vals
    return np.ascontiguousarray(arr.reshape(-1, 16).T)     # [16, ntiles*8]


def _prepare(inputs):
    x = np.asarray(inputs["x"])
    nt = np.asarray(inputs["node_type"]).astype(np.int64)
    item = int(np.asarray(inputs["item_id"]))
    emb = np.asarray(inputs["emb_weight"], dtype=np.float32)
    W = np.asarray(inputs["W"], dtype=np.float32)
    b = np.asarray(inputs["b"], dtype=np.float32)

    t2 = np.ascontiguousarray(np.broadcast_to(emb.astype(BF16), (2, NUM_T, HID)))
    wt = np.ascontiguousarray(W.T).astype(BF16)
    bb = b.astype(np.float32).reshape(HID, 1)

    sel_los, sel_his = [], []
    max_lo = max_hi = 0
    for c in range(NCORES):
        sel = np.flatnonzero(nt[c * NSH : (c + 1) * NSH] == item)
        sel_lo = sel[sel < HALF].astype(np.int32)
        sel_hi = (sel[sel >= HALF] - HALF).astype(np.int32)
        sel_los.append(sel_lo)
        sel_his.append(sel_hi)
        max_lo = max(max_lo, len(sel_lo))
        max_hi = max(max_hi, len(sel_hi))
    # tiles per region, multiples of 8 (whole GCHUNK calls), with headroom
    tlo = max(32, -(-max_lo // 128) + 7 & ~7)
    thi = max(32, -(-max_hi // 128) + 7 & ~7)

    in_maps = []
    for c in range(NCORES):
        nt_sh = nt[c * NSH : (c + 1) * NSH]
        xb = x[c * NSH : (c + 1) * NSH].astype(BF16)

        oh1 = np.zeros((NUM_T, PADR), dtype=BF16)
        for t in range(NUM_T):
            oh1[t, :NSH] = (nt_sh == t)
        oh = np.ascontiguousarray(np.broadcast_to(oh1, (2, NUM_T, PADR)))

        idx16 = np.concatenate(
            [_pack_idx16(sel_los[c], tlo), _pack_idx16(sel_his[c], thi)], axis=1)
        idx16 = np.ascontiguousarray(np.tile(idx16, (8, 1)))   # [128, tsel*8]

        in_maps.append({"xlo": np.ascontiguousarray(xb[:HALF]),
                        "xhi": np.ascontiguousarray(xb[HALF:]), "oh": oh, "idx": idx16,
                        "t2": t2, "wt": wt, "bb": bb,
                        "ident": np.eye(128, dtype=BF16)})
    return tlo, thi, sel_los, sel_his, in_maps


def _run(inputs, trace=False):
    _ensure_axon_profile_hook()
    tlo, thi, sel_los, sel_his, in_maps = _prepare(inputs)
    if (tlo, thi) not in _CACHE:
        _CACHE[(tlo, thi)] = _build(tlo, thi)
    nc = _CACHE[(tlo, thi)]
    res = run_bass_kernel_spmd(nc, in_maps, core_ids=list(range(NCORES)), trace=trace)
    out = np.empty((N, HID), np.float32)
    for c in range(NCORES):
        outT = res.results[c]["outT"]          # [HID, PADR] bf16
        osh = out[c * NSH : (c + 1) * NSH]
        osh[:] = outT[:, :NSH].astype(np.float32).T
        o2T = res.results[c]["o2T"]            # [HID, tsel*128] bf16
        lo, hi = sel_los[c], sel_his[c]
        if len(lo):
            osh[lo] = o2T[:, : len(lo)].astype(np.float32).T
        if len(hi):
            base = tlo * 128
            osh[hi + HALF] = o2T[:, base : base + len(hi)].astype(np.float32).T
    return out, res


def kernel(**inputs) -> np.ndarray:
    out, _ = _run(inputs, trace=bool(os.environ.get("KERNEL_TRACE")))
    return out


# revision 29
# speedup vs baseline: 1.1054x; 1.0582x over previous
"""Trainium2 Bass kernel for nn_Node_Transformation.

Reference semantics, for row n:
    out[n] = x[n] @ W.T + b            if node_type[n] == item_id
             emb_weight[node_type[n]]  otherwise

Only ~1/8 of rows take the linear path, so the kernel is split:

  Dense part (all rows): out_dense[n] = emb_weight[node_type[n]], computed as a
  one-hot matmul: outT[h, r] = sum_t table[t, h] * onehot[t, r], with the tiny
  table as the stationary operand and the host-built one-hot indicator
  streaming as rhs. Output is produced hid-major ("outT") so the per-group
  varying operand is the streaming one (no LDWEIGHTS churn).

  Sparse part (selected rows only): row indices where node_type == item_id are
  computed on host (metadata only); the kernel bulk-gathers just those x rows
  with dma_gather (1/8 of x traffic, ~1 Q7 descriptor-gen call per 1024 rows),
  transposes them on the PE, and computes lin = x_sel @ W.T + b into a compact
  second output. The host scatters those rows over the dense result while
  unsharding. dma_gather needs int16 indices, so each shard's x is staged as
  two half tensors (rows < 31250 and >= 31250) with rebased indices; slots are
  padded with index 0 so num_idxs is the same on every core (SPMD).

Everything on-device is bf16 (psum accumulation in f32); the correctness gate
is a scale-relative 2e-2 absmax, bf16 error is ~4e-3.

Sharding: data-parallel over N across 8 NeuronCores; weights/table replicated.
"""

import os
import numpy as np
import ml_dtypes

import concourse.bass as bass
import concourse.bacc as bacc
import concourse.mybir as mybir
from concourse.tile import TileContext
from concourse.bass_utils import run_bass_kernel_spmd
from bass_rust import add_dep_helper

# ---- problem constants (hardcoded per contest contract) ----
N = 500000
IN_CH = 256
HID = 128
NUM_T = 8
NCORES = 8
NSH = N // NCORES          # 62500 rows per core
HALF = NSH // 2            # 31250: x split so gather indices fit int16
DGRP = 512                 # dense rows per matmul group (one f32 PSUM bank)
NG = (NSH + DGRP - 1) // DGRP          # 123 dense groups
PADR = NG * DGRP                       # 62976 padded rows per core
OHC = 8192                 # one-hot columns loaded per DMA (16 dense groups)
SLABG = 16                 # dense groups per output slab (8192 cols per DMA)
SGRP = 512                 # sel rows per matmul group (4 tiles, f32 psum bank)
GCHUNK = 1024              # sel rows per dma_gather call (2 matmul groups)

BF16 = ml_dtypes.bfloat16

_CACHE = {}


def _ensure_axon_profile_hook():
    """bass_utils' trace path imports antenv.axon_hooks, which this image
    lacks. Register an equivalent module backed by the axon PJRT .so so
    trace=True (or BASS_TRACE=1) works instead of crashing."""
    try:
        import antenv.axon_hooks  # noqa: F401
        return
    except ImportError:
        pass
    import sys
    import types

    hook = None
    try:
        from trn_agent_boot.trn_boot import _ntff_profile_via_ctypes

        hook = _ntff_profile_via_ctypes("/opt/axon/libaxon_pjrt.so")
    except Exception:
        hook = None
    mod = types.ModuleType("antenv.axon_hooks")
    mod.get_axon_ntff_profile_hook = lambda: hook
    mod.set_axon_ntff_profile_hook = lambda h: None
    sys.modules["antenv.axon_hooks"] = mod
    try:
        import antenv

        antenv.axon_hooks = mod
    except ImportError:
        pass


def _build(tlo: int, thi: int) -> bass.Bass:
    """tlo/thi: number of 128-row sel tiles gathered from the low/high half
    of x. Both are multiples of 8 so gathers are whole GCHUNK calls."""
    # The stock cost model grossly underestimates SWDGE descriptor-gen time
    # (0.34 ns/descriptor; ~7.5 ns measured on hardware), which makes the
    # tile scheduler place gather-dependent work far too early. Correct it
    # for the duration of this build, then restore.
    from concourse import hw_specs as _hw
    _saved = _hw.TRN2Spec.SWDGE_NS_PER_DESCRIPTOR
    _hw.TRN2Spec.SWDGE_NS_PER_DESCRIPTOR = 7.5
    try:
        return _build_inner(tlo, thi)
    finally:
        _hw.TRN2Spec.SWDGE_NS_PER_DESCRIPTOR = _saved


def _build_inner(tlo: int, thi: int) -> bass.Bass:
    nc = bacc.Bacc("TRN2")
    f32 = mybir.dt.float32
    bf16 = mybir.dt.bfloat16
    i16 = mybir.dt.int16

    tsel = tlo + thi
    ncalls = tsel * 128 // GCHUNK
    calls_lo = tlo * 128 // GCHUNK
    sgroups = tsel * 128 // SGRP

    xlo_d = nc.dram_tensor("xlo", [HALF, IN_CH], bf16, kind="ExternalInput")
    xhi_d = nc.dram_tensor("xhi", [HALF, IN_CH], bf16, kind="ExternalInput")
    id_d = nc.dram_tensor("ident", [128, 128], bf16, kind="ExternalInput")
    oh_d = nc.dram_tensor("oh", [2, NUM_T, PADR], bf16, kind="ExternalInput")
    idx_d = nc.dram_tensor("idx", [128, tsel * 8], i16, kind="ExternalInput")
    t2_d = nc.dram_tensor("t2", [2, NUM_T, HID], bf16, kind="ExternalInput")
    wt_d = nc.dram_tensor("wt", [IN_CH, HID], bf16, kind="ExternalInput")
    bb_d = nc.dram_tensor("bb", [HID, 1], f32, kind="ExternalInput")
    outT_d = nc.dram_tensor("outT", [HID, PADR], bf16, kind="ExternalOutput")
    o2T_d = nc.dram_tensor("o2T", [HID, tsel * 128], bf16, kind="ExternalOutput")

    # spread the sel compute groups through the dense loop, starting late
    # enough that the first gather (Q7 descriptor-gen ~9us) has really landed
    first_t = 28
    sel_at = sorted(set(first_t + int(round(i * (NG - 2 - first_t) / max(1, sgroups - 1)))
                        for i in range(sgroups)))
    assert len(sel_at) == sgroups

    with TileContext(nc) as tc:
        with (
            tc.tile_pool(name="singles", bufs=1) as singles,
            tc.tile_pool(name="ohp", bufs=3) as ohpool,
            tc.tile_pool(name="osl", bufs=4) as opool,
            tc.tile_pool(name="xsp", bufs=4) as xpool,
            tc.tile_pool(name="xtp", bufs=3) as xtpool,
            tc.tile_pool(name="o2p", bufs=3) as o2pool,
            tc.tile_pool(name="psd", bufs=5, space="PSUM") as psd,
            tc.tile_pool(name="pst", bufs=2, space="PSUM") as pst,
            tc.tile_pool(name="psl", bufs=1, space="PSUM") as psl,
        ):
            oh_tiles = {}
            oslab = None
            slab_g0 = 0
            gathered = {}          # call index -> xg slab tile

            def emit_gather(k):
                if k >= ncalls or k in gathered:
                    return
                xg = xpool.tile([128, GCHUNK // 128, IN_CH], bf16, tag="xg")
                src_ap = (xlo_d if k < calls_lo else xhi_d)[:]
                cols = GCHUNK // 16
                nc.gpsimd.dma_gather(
                    out_ap=xg[:],
                    in_ap=src_ap,
                    idxs_ap=idx_s[:, k * cols : (k + 1) * cols],
                    num_idxs=GCHUNK,
                    num_idxs_reg=GCHUNK,
                    elem_size=IN_CH,
                )
                gathered[k] = xg

            def emit_sel_compute(sg, anchor):
                k = sg * SGRP // GCHUNK
                j0 = (sg * SGRP - k * GCHUNK) // 128
                xg = gathered[k]
                xsT = xtpool.tile([128, 2, SGRP], bf16, tag="xsT")
                for j in range(SGRP // 128):
                    pt = pst.tile([128, 2, 128], bf16, tag="pt")
                    t1 = nc.tensor.transpose(pt[:, 0, :], xg[:, j0 + j, 0:128], ident[:])
                    t2 = nc.tensor.transpose(pt[:, 1, :], xg[:, j0 + j, 128:256], ident[:])
                    if anchor is not None:
                        add_dep_helper(t1.ins, anchor.ins, sync=False,
                                       reason="defer sel transposes behind dense")
                    nc.vector.tensor_copy(xsT[:, :, j * 128 : (j + 1) * 128], pt[:])
                if j0 + SGRP // 128 >= GCHUNK // 128:
                    del gathered[k]
                lp = psl.tile([HID, SGRP], f32, tag="lp")
                nc.tensor.matmul(out=lp[:], lhsT=wt_s[:, 0, :], rhs=xsT[:, 0, :],
                                 start=True, stop=False)
                nc.tensor.matmul(out=lp[:], lhsT=wt_s[:, 1, :], rhs=xsT[:, 1, :],
                                 start=False, stop=True)
                o2 = o2pool.tile([HID, SGRP], bf16, tag="o2")
                nc.scalar.activation(out=o2[:], in_=lp[:],
                                     func=mybir.ActivationFunctionType.Identity,
                                     bias=bb_s[:, 0:1], scale=1.0)
                nc.scalar.dma_start(out=o2T_d[:, sg * SGRP : (sg + 1) * SGRP], in_=o2[:])

            def load_oh_chunk(ci):
                if ci * OHC >= PADR or ci in oh_tiles:
                    return
                tile = ohpool.tile([64, OHC], bf16, tag="oh")
                lo = ci * OHC
                hi = min(lo + OHC, PADR)
                for a in range(2):
                    nc.sync.dma_start(out=tile[32 * a : 32 * a + NUM_T, 0 : hi - lo],
                                      in_=oh_d[a, :, lo:hi])
                oh_tiles[ci] = tile

            # prologue: gather indices and t2 first (gathers and the dense
            # path are the two long poles), then oh chunks, then other consts
            idx_s = singles.tile([128, tsel * 8], i16)
            nc.sync.dma_start(out=idx_s[:], in_=idx_d[:])
            t2_s = singles.tile([64, HID], bf16)
            for a in range(2):
                nc.sync.dma_start(out=t2_s[32 * a : 32 * a + NUM_T, :], in_=t2_d[a])
            load_oh_chunk(0)
            emit_gather(0)
            load_oh_chunk(1)
            ident = singles.tile([128, 128], bf16)
            nc.sync.dma_start(out=ident[:], in_=id_d[:])
            wt_s = singles.tile([128, 2, HID], bf16)
            nc.sync.dma_start(out=wt_s[:], in_=wt_d[:].rearrange("(k c) h -> c k h", k=2))
            bb_s = singles.tile([HID, 1], f32)
            nc.sync.dma_start(out=bb_s[:], in_=bb_d[:])
            emit_gather(1)
            emit_gather(2)

            sel_i = 0
            for g in range(NG):
                c0 = g * DGRP
                ci = g // (OHC // DGRP)
                if g % (OHC // DGRP) == 0:
                    load_oh_chunk(ci + 2)
                if g % SLABG == 0:
                    oslab = opool.tile([HID, SLABG * DGRP], bf16, tag="oslab")
                    slab_g0 = g

                j = g % 2
                pd = psd.tile([HID, DGRP], f32, tag="pd")
                last_mm = nc.tensor.matmul(
                    out=pd[:], lhsT=t2_s[32 * j : 32 * j + NUM_T, :],
                    rhs=oh_tiles[ci][32 * j : 32 * j + NUM_T,
                                     c0 - ci * OHC : c0 - ci * OHC + DGRP],
                    start=True, stop=True,
                    tile_position=(32 * j, 0),
                )
                so = (g - slab_g0) * DGRP
                if g % 5 < 3:
                    nc.vector.tensor_copy(oslab[:, so : so + DGRP], pd[:])
                else:
                    nc.scalar.activation(out=oslab[:, so : so + DGRP], in_=pd[:],
                                         func=mybir.ActivationFunctionType.Copy)

                if g == slab_g0 + SLABG - 1 or g == NG - 1:
                    lo = slab_g0 * DGRP
                    hi = (g + 1) * DGRP
                    nc.scalar.dma_start(out=outT_d[:, lo:hi], in_=oslab[:, 0 : hi - lo])

                if sel_i < sgroups and g == sel_at[sel_i]:
                    if sel_i % (GCHUNK // SGRP) == 0:
                        emit_gather(sel_i * SGRP // GCHUNK + 3)
                    emit_sel_compute(sel_i, last_mm)
                    sel_i += 1

            while sel_i < sgroups:
                if sel_i % (GCHUNK // SGRP) == 0:
                    emit_gather(sel_i * SGRP // GCHUNK + 3)
                emit_sel_compute(sel_i, None)
                sel_i += 1

    nc.compile()
    return nc


def _pack_idx16(vals: np.ndarray, ntiles: int) -> np.ndarray:
    """Pack a region's rebased indices (padded with 0) into the dma_gather
    index layout: slot i -> [i % 16, i // 16], 16-partition pattern."""
    arr = np.zeros(ntiles * 128, dtype=np.int16)
    arr[: len(vals)] = vals
    return np.ascontiguousarray(arr.reshape(-1, 16).T)     # [16, ntiles*8]


def _prepare(inputs):
    x = np.asarray(inputs["x"])
    nt = np.asarray(inputs["node_type"]).astype(np.int64)
    item = int(np.asarray(inputs["item_id"]))
    emb = np.asarray(inputs["emb_weight"], dtype=np.float32)
    W = np.asarray(inputs["W"], dtype=np.float32)
    b = np.asarray(inputs["b"], dtype=np.float32)

    t2 = np.ascontiguousarray(np.broadcast_to(emb.astype(BF16), (2, NUM_T, HID)))
    wt = np.ascontiguousarray(W.T).astype(BF16)
    bb = b.astype(np.float32).reshape(HID, 1)

    sel_los, sel_his = [], []
    max_lo = max_hi = 0
    for c in range(NCORES):
        sel = np.flatnonzero(nt[c * NSH : (c + 1) * NSH] == item)
        sel_lo = sel[sel < HALF].astype(np.int32)
        sel_hi = (sel[sel >= HALF] - HALF).astype(np.int32)
        sel_los.append(sel_lo)
        sel_his.append(sel_hi)
        max_lo = max(max_lo, len(sel_lo))
        max_hi = max(max_hi, len(sel_hi))
    # tiles per region, multiples of 8 (whole GCHUNK calls), with headroom
    tlo = max(32, -(-max_lo // 128) + 7 & ~7)
    thi = max(32, -(-max_hi // 128) + 7 & ~7)

    in_maps = []
    for c in range(NCORES):
        nt_sh = nt[c * NSH : (c + 1) * NSH]
        xb = x[c * NSH : (c + 1) * NSH].astype(BF16)

        oh1 = np.zeros((NUM_T, PADR), dtype=BF16)
        for t in range(NUM_T):
            oh1[t, :NSH] = (nt_sh == t)
        oh = np.ascontiguousarray(np.broadcast_to(oh1, (2, NUM_T, PADR)))

        idx16 = np.concatenate(
            [_pack_idx16(sel_los[c], tlo), _pack_idx16(sel_his[c], thi)], axis=1)
        idx16 = np.ascontiguousarray(np.tile(idx16, (8, 1)))   # [128, tsel*8]

        in_maps.append({"xlo": np.ascontiguousarray(xb[:HALF]),
                        "xhi": np.ascontiguousarray(xb[HALF:]), "oh": oh, "idx": idx16,
                        "t2": t2, "wt": wt, "bb": bb,
                        "ident": np.eye(128, dtype=BF16)})
    return tlo, thi, sel_los, sel_his, in_maps


def _run(inputs, trace=False):
    _ensure_axon_profile_hook()
    tlo, thi, sel_los, sel_his, in_maps = _prepare(inputs)
    if (tlo, thi) not in _CACHE:
        _CACHE[(tlo, thi)] = _build(tlo, thi)
    nc = _CACHE[(tlo, thi)]
    res = run_bass_kernel_spmd(nc, in_maps, core_ids=list(range(NCORES)), trace=trace)
    out = np.empty((N, HID), np.float32)
    for c in range(NCORES):
        outT = res.results[c]["outT"]          # [HID, PADR] bf16
        osh = out[c * NSH : (c + 1) * NSH]
        osh[:] = outT[:, :NSH].astype(np.float32).T
        o2T = res.results[c]["o2T"]            # [HID, tsel*128] bf16
        lo, hi = sel_los[c], sel_his[c]
        if len(lo):
            osh[lo] = o2T[:, : len(lo)].astype(np.float32).T
        if len(hi):
            base = tlo * 128
            osh[hi + HALF] = o2T[:, base : base + len(hi)].astype(np.float32).T
    return out, res


def kernel(**inputs) -> np.ndarray:
    out, _ = _run(inputs, trace=bool(os.environ.get("KERNEL_TRACE")))
    return out
